# revision 10
# baseline (speedup 1.0000x reference)
"""Self-attention scores kernel for Trainium2, 8-core SPMD. (v3)

Computes softmax((x@Wq+bq) @ (x@Wq+bq)^T / sqrt(64)) per head
(reference reuses the query projection for k, bug-for-bug).

Sharding: 32 (batch, head) pairs split 4-per-core across 8 cores.
Core c handles batch c//4, heads 4*(c%4) .. 4*(c%4)+3.

v3 structure (baseline was 172us, all exp on ACT, PE HAM-throttled):
- Heads are processed in pairs: head 2p uses q rows in partitions 0-63,
  head 2p+1 in 64-127.  Their 64-contraction score matmuls target
  different PE row-groups (tile_position inferred from base partition),
  so consecutive slots' matmuls execute CONCURRENTLY in the two array
  halves.  Even with HAM at K=4/8 (1.2 GHz) the PE streams a [128,2048]
  block in ~0.9us -- below the consumer rate -- so no junk matmuls or
  HAM games are needed.
- The exp is split: most blocks on ACT (Exp, N=2048, free row-sum
  accumulator); every ~5th block on DVE via a Schraudolph bit-trick:
  int16(round(A*e + B)) IS the bf16 bit pattern of exp(e/8) (~3%
  sawtooth error that mostly cancels after row normalization; measured
  ~7e-3 l2 worst case, 4e-3 mixed).  DVE blocks get their row-sum from
  an in-place copy-with-accumulator pass.
- Row reciprocals batched per 4-block group; row-scales on DVE (4x
  bf16).  GPSIMD does no compute (its tensor ops are ~29us AND stall
  DVE via the shared SBUF port); it only runs the SWDGE DMA ring.
- Wq is pre-swizzled on the host so its DMA is contiguous.
"""

import numpy as np

import concourse.bass as bass
import concourse.mybir as mybir
import concourse.tile as tile
from concourse import bacc
from concourse.bass_utils import run_bass_kernel_spmd

B = 2
S = 2048
D = 1024
H = 16
HS = 64
N_CORES = 8
HEADS_PER_CORE = 4
KK = D // 128  # k-tiles for the projection contraction
NQ = S // 128  # 16 q row-blocks per head
GRP = 4  # row-blocks batched per output DMA (2 MiB)

MM_DT = mybir.dt.bfloat16
IN_DT = mybir.dt.float16
OUT_DT = mybir.dt.bfloat16
F32 = mybir.dt.float32
I16 = mybir.dt.int16

# Schraudolph exp: bits = round(A*e + B) interpreted as bf16 = exp(e/8).
A_EXP = float((2 ** 7) * np.log2(np.e) / 8.0)
B_EXP = float(127.0 * 2 ** 7 - 0.5)

# Slots (i*2+hh for block i, head-half hh) whose exp runs on DVE.
DVE_SLOTS = frozenset({2, 7, 12, 17, 22, 27})


def _build():
    nc = bacc.Bacc("TRN2", target_bir_lowering=False, debug=False)
    xT = nc.dram_tensor("xT", [D, S], IN_DT, kind="ExternalInput").ap()
    WqS = nc.dram_tensor("WqS", [128, KK * HEADS_PER_CORE * HS], IN_DT, kind="ExternalInput").ap()
    bqS = nc.dram_tensor("bqS", [128, 2], F32, kind="ExternalInput").ap()
    out = nc.dram_tensor("out", [HEADS_PER_CORE, S, S], OUT_DT, kind="ExternalOutput").ap()

    with tile.TileContext(nc) as tc:
        with (
            tc.tile_pool(name="consts", bufs=1) as consts,
            tc.tile_pool(name="xt", bufs=KK) as xt_pool,
            tc.tile_pool(name="et", bufs=5) as et_pool,
            tc.tile_pool(name="small", bufs=8) as small,
        ):
            w = consts.tile([128, KK, HEADS_PER_CORE * HS], IN_DT)
            nc.scalar.dma_start(out=w[:], in_=WqS)
            bias = consts.tile([128, 2], F32)
            nc.scalar.dma_start(out=bias[:], in_=bqS)

            xts = []
            for kk in range(KK):
                xtt = xt_pool.tile([128, S], IN_DT, tag="xt")
                nc.sync.dma_start(out=xtt[:], in_=xT[kk * 128 : (kk + 1) * 128, :])
                xts.append(xtt)

            # ---- Projection (all 8 PSUM banks; pool closes after) ----
            qts = []
            with tc.tile_pool(name="proj_ps", bufs=2, space="PSUM") as proj_ps:
                psA = proj_ps.tile([128, S], F32, tag="ps", name="psA")
                psB = proj_ps.tile([128, S], F32, tag="ps", name="psB")
                pss = [psA, psB]
                for kk in range(KK):
                    for g in range(2):
                        for n in range(4):
                            nc.tensor.matmul(
                                pss[g][:, n * 512 : (n + 1) * 512],
                                lhsT=w[:, kk, g * 128 : (g + 1) * 128],
                                rhs=xts[kk][:, n * 512 : (n + 1) * 512],
                                start=(kk == 0),
                                stop=(kk == KK - 1),
                            )
                for g in range(2):
                    qtg = consts.tile([128, S], MM_DT, tag=f"qt{g}", name=f"qt{g}")
                    nc.vector.tensor_scalar_add(qtg[:], pss[g][:], bias[:, g : g + 1])
                    qts.append(qtg)

            # ---- Scores + softmax, head-pair concurrent ----
            with tc.tile_pool(name="ps", bufs=2, space="PSUM") as ps_pool:
                for p in range(2):  # head pair
                    qtg = qts[p]
                    last_pair = p == 1
                    for grp in range(NQ // GRP):
                        ets = []
                        rsgs = []
                        dve_pend = []
                        for hh in range(2):
                            ets.append(
                                et_pool.tile(
                                    [128, GRP, S], OUT_DT, tag="et",
                                    name=f"et_p{p}g{grp}h{hh}",
                                )
                            )
                            rsgs.append(
                                small.tile(
                                    [128, GRP], F32, tag="rs",
                                    name=f"rs_p{p}g{grp}h{hh}",
                                )
                            )
                        for q in range(GRP):
                            i = grp * GRP + q
                            for hh in range(2):
                                pb = 64 * hh
                                lhsT = qtg[pb : pb + 64, i * 128 : (i + 1) * 128]
                                ps = ps_pool.tile([128, S], F32, tag="ps")
                                for n in range(4):
                                    nc.tensor.matmul(
                                        ps[:, n * 512 : (n + 1) * 512],
                                        lhsT=lhsT,
                                        rhs=qtg[pb : pb + 64, n * 512 : (n + 1) * 512],
                                        start=True,
                                        stop=True,
                                    )
                                et = ets[hh]
                                rsg = rsgs[hh]
                                if (i * 2 + hh) in DVE_SLOTS:
                                    nc.vector.tensor_scalar(
                                        et[:, q, :].bitcast(I16),
                                        ps[:],
                                        A_EXP,
                                        B_EXP,
                                        mybir.AluOpType.mult,
                                        mybir.AluOpType.add,
                                    )
                                    # row-sum deferred to the group epilogue
                                    # so this slot's PSUM frees immediately.
                                    dve_pend.append((et, q, rsg))
                                else:
                                    nc.scalar.activation(
                                        out=et[:, q, :],
                                        in_=ps[:],
                                        func=mybir.ActivationFunctionType.Exp,
                                        scale=1.0 / np.sqrt(float(HS)),
                                        accum_out=rsg[:, q : q + 1],
                                    )
                        # group epilogue: deferred DVE row-sums, then per head
                        # one reciprocal, 4 scales, DMA
                        for et, q, rsg in dve_pend:
                            nc.vector.tensor_scalar(
                                et[:, q, :],
                                et[:, q, :],
                                1.0,
                                0.0,
                                mybir.AluOpType.mult,
                                mybir.AluOpType.add,
                                accum_out=rsg[:, q : q + 1],
                            )
                        for hh in range(2):
                            h = 2 * p + hh
                            et, rsg = ets[hh], rsgs[hh]
                            rcg = small.tile([128, GRP], F32, tag="rc")
                            nc.vector.reciprocal(rcg[:], rsg[:])
                            for q in range(GRP):
                                nc.vector.tensor_scalar_mul(
                                    et[:, q, :], et[:, q, :], rcg[:, q : q + 1]
                                )
                                if last_pair and grp == NQ // GRP - 1:
                                    i = grp * GRP + q
                                    eng = nc.sync if (q * 2 + hh) % 2 == 0 else nc.gpsimd
                                    eng.dma_start(
                                        out=out[h, i * 128 : (i + 1) * 128, :],
                                        in_=et[:, q, :],
                                    )
                            if last_pair and grp == NQ // GRP - 1:
                                continue
                            eng = nc.sync if (grp * 2 + hh) % 2 == 0 else nc.gpsimd
                            eng.dma_start(
                                out=out[
                                    h, grp * GRP * 128 : (grp + 1) * GRP * 128, :
                                ].rearrange("(c p) s -> p c s", p=128),
                                in_=et[:],
                            )
    nc.compile()
    return nc


_NC_CACHE = None


def kernel(x, Wq, bq):
    global _NC_CACHE
    x = np.asarray(x, dtype=np.float32)
    Wq = np.asarray(Wq, dtype=np.float32)
    bq = np.asarray(bq, dtype=np.float32)
    assert x.shape == (B, S, D) and Wq.shape == (D, D) and bq.shape == (D,)

    if _NC_CACHE is None:
        _NC_CACHE = _build()
    nc = _NC_CACHE

    xTs = [np.ascontiguousarray(x[b].T.astype(np.float16)) for b in range(B)]
    Wq16 = Wq.astype(np.float16)
    in_maps = []
    for c in range(N_CORES):
        b, hg = divmod(c, N_CORES // B)
        h0 = hg * HEADS_PER_CORE
        wslice = Wq16[:, h0 * HS : (h0 + HEADS_PER_CORE) * HS]  # [1024, 256]
        wsw = np.ascontiguousarray(
            wslice.reshape(KK, 128, HEADS_PER_CORE * HS)
            .transpose(1, 0, 2)
            .reshape(128, KK * HEADS_PER_CORE * HS)
        )
        in_maps.append(
            {
                "xT": xTs[b],
                "WqS": wsw,
                "bqS": np.ascontiguousarray(
                    bq[h0 * HS : (h0 + HEADS_PER_CORE) * HS].reshape(2, 128).T
                ),
            }
        )

    res = run_bass_kernel_spmd(nc, in_maps, core_ids=list(range(N_CORES)))

    full = np.empty((B, H, S, S), dtype=np.float32)
    for c in range(N_CORES):
        b, hg = divmod(c, N_CORES // B)
        h0 = hg * HEADS_PER_CORE
        full[b, h0 : h0 + HEADS_PER_CORE] = np.asarray(
            res.results[c]["out"]
        ).astype(np.float32)
    return full


# revision 14
# speedup vs baseline: 1.2193x; 1.2193x over previous
"""Self-attention scores kernel for Trainium2, 8-core SPMD. (v3)

Computes softmax((x@Wq+bq) @ (x@Wq+bq)^T / sqrt(64)) per head
(reference reuses the query projection for k, bug-for-bug).

Sharding: 32 (batch, head) pairs split 4-per-core across 8 cores.
Core c handles batch c//4, heads 4*(c%4) .. 4*(c%4)+3.

v3 structure (baseline was 172us, all exp on ACT, PE HAM-throttled):
- Heads are processed in pairs: head 2p uses q rows in partitions 0-63,
  head 2p+1 in 64-127.  Their 64-contraction score matmuls target
  different PE row-groups (tile_position inferred from base partition),
  so consecutive slots' matmuls execute CONCURRENTLY in the two array
  halves.  Even with HAM at K=4/8 (1.2 GHz) the PE streams a [128,2048]
  block in ~0.9us -- below the consumer rate -- so no junk matmuls or
  HAM games are needed.
- The exp is split: most blocks on ACT (Exp, N=2048, free row-sum
  accumulator); every ~5th block on DVE via a Schraudolph bit-trick:
  int16(round(A*e + B)) IS the bf16 bit pattern of exp(e/8) (~3%
  sawtooth error that mostly cancels after row normalization; measured
  ~7e-3 l2 worst case, 4e-3 mixed).  DVE blocks get their row-sum from
  an in-place copy-with-accumulator pass.
- Row reciprocals batched per 4-block group; row-scales on DVE (4x
  bf16).  GPSIMD does no compute (its tensor ops are ~29us AND stall
  DVE via the shared SBUF port); it only runs the SWDGE DMA ring.
- Wq is pre-swizzled on the host so its DMA is contiguous.
"""

import numpy as np

import concourse.bass as bass
import concourse.mybir as mybir
import concourse.tile as tile
from concourse import bacc
from concourse.bass_utils import run_bass_kernel_spmd

B = 2
S = 2048
D = 1024
H = 16
HS = 64
N_CORES = 8
HEADS_PER_CORE = 4
KK = D // 128  # k-tiles for the projection contraction
NQ = S // 128  # 16 q row-blocks per head
GRP = 4  # row-blocks batched per output DMA (2 MiB)

MM_DT = mybir.dt.bfloat16
IN_DT = mybir.dt.float16
OUT_DT = mybir.dt.bfloat16
F32 = mybir.dt.float32
I16 = mybir.dt.int16

# Schraudolph exp: bits = round(A*e + B) interpreted as bf16 = exp(e/8).
A_EXP = float((2 ** 7) * np.log2(np.e) / 8.0)
B_EXP = float(127.0 * 2 ** 7 - 0.5)

# Slots (i*2+hh for block i, head-half hh) whose exp runs on DVE.
DVE_SLOTS = frozenset({2, 8, 14, 20, 26})


def _build():
    nc = bacc.Bacc("TRN2", target_bir_lowering=False, debug=False)
    xT = nc.dram_tensor("xT", [D, S], IN_DT, kind="ExternalInput").ap()
    WqS = nc.dram_tensor("WqS", [128, KK * HEADS_PER_CORE * HS], IN_DT, kind="ExternalInput").ap()
    bqS = nc.dram_tensor("bqS", [128, 2], F32, kind="ExternalInput").ap()
    out = nc.dram_tensor("out", [HEADS_PER_CORE, S, S], OUT_DT, kind="ExternalOutput").ap()

    with tile.TileContext(nc) as tc:
        with (
            tc.tile_pool(name="consts", bufs=1) as consts,
            tc.tile_pool(name="et", bufs=8) as et_pool,
            tc.tile_pool(name="small", bufs=12) as small,
        ):
            w = consts.tile([128, KK, HEADS_PER_CORE * HS], IN_DT)
            nc.scalar.dma_start(out=w[:], in_=WqS)
            bias = consts.tile([128, 2], F32)
            nc.scalar.dma_start(out=bias[:], in_=bqS)

            # ---- Projection (all 8 PSUM banks; pools close after) ----
            qts = []
            with (
                tc.tile_pool(name="xt", bufs=KK) as xt_pool,
                tc.tile_pool(name="proj_ps", bufs=2, space="PSUM") as proj_ps,
            ):
                xts = []
                for kk in range(KK):
                    xtt = xt_pool.tile([128, S], IN_DT, tag="xt")
                    nc.sync.dma_start(out=xtt[:], in_=xT[kk * 128 : (kk + 1) * 128, :])
                    xts.append(xtt)
                psA = proj_ps.tile([128, S], F32, tag="ps", name="psA")
                psB = proj_ps.tile([128, S], F32, tag="ps", name="psB")
                pss = [psA, psB]
                for kk in range(KK):
                    for g in range(2):
                        for n in range(4):
                            nc.tensor.matmul(
                                pss[g][:, n * 512 : (n + 1) * 512],
                                lhsT=w[:, kk, g * 128 : (g + 1) * 128],
                                rhs=xts[kk][:, n * 512 : (n + 1) * 512],
                                start=(kk == 0),
                                stop=(kk == KK - 1),
                            )
                for g in range(2):
                    qtg = consts.tile([128, S], MM_DT, tag=f"qt{g}", name=f"qt{g}")
                    nc.vector.tensor_scalar_add(qtg[:], pss[g][:], bias[:, g : g + 1])
                    qts.append(qtg)

            # ---- Scores + softmax, head-pair concurrent ----
            # The group epilogue (deferred DVE row-sums, reciprocal, scales,
            # DMA) is software-pipelined ONE GROUP BEHIND the exp stream so
            # the DVE epilogue chain never blocks the next group's exps.
            def emit_epilogue(state, per_block_dma):
                p, grp, ets, rsgs, dve_pend = state
                for et, q, rsg in dve_pend:
                    nc.vector.tensor_scalar(
                        et[:, q, :], et[:, q, :], 1.0, 0.0,
                        mybir.AluOpType.mult, mybir.AluOpType.add,
                        accum_out=rsg[:, q : q + 1],
                    )
                for hh in range(2):
                    h = 2 * p + hh
                    et, rsg = ets[hh], rsgs[hh]
                    rcg = small.tile(
                        [128, GRP], F32, tag="rc", name=f"rc_p{p}g{grp}h{hh}"
                    )
                    nc.vector.reciprocal(rcg[:], rsg[:])
                    for q in range(GRP):
                        nc.vector.tensor_scalar_mul(
                            et[:, q, :], et[:, q, :], rcg[:, q : q + 1]
                        )
                        if per_block_dma:
                            i = grp * GRP + q
                            eng = nc.sync if (q * 2 + hh) % 2 == 0 else nc.gpsimd
                            eng.dma_start(
                                out=out[h, i * 128 : (i + 1) * 128, :],
                                in_=et[:, q, :],
                            )
                    if per_block_dma:
                        continue
                    eng = nc.sync if (grp * 2 + hh) % 2 == 0 else nc.gpsimd
                    eng.dma_start(
                        out=out[
                            h, grp * GRP * 128 : (grp + 1) * GRP * 128, :
                        ].rearrange("(c p) s -> p c s", p=128),
                        in_=et[:],
                    )

            with tc.tile_pool(name="ps", bufs=2, space="PSUM") as ps_pool:
                pending = None
                for p in range(2):  # head pair
                    qtg = qts[p]
                    for grp in range(NQ // GRP):
                        ets = []
                        rsgs = []
                        dve_pend = []
                        for hh in range(2):
                            ets.append(
                                et_pool.tile(
                                    [128, GRP, S], OUT_DT, tag="et",
                                    name=f"et_p{p}g{grp}h{hh}",
                                )
                            )
                            rsgs.append(
                                small.tile(
                                    [128, GRP], F32, tag="rs",
                                    name=f"rs_p{p}g{grp}h{hh}",
                                )
                            )
                        for q in range(GRP):
                            i = grp * GRP + q
                            for hh in range(2):
                                pb = 64 * hh
                                lhsT = qtg[pb : pb + 64, i * 128 : (i + 1) * 128]
                                ps = ps_pool.tile([128, S], F32, tag="ps")
                                for n in range(4):
                                    nc.tensor.matmul(
                                        ps[:, n * 512 : (n + 1) * 512],
                                        lhsT=lhsT,
                                        rhs=qtg[pb : pb + 64, n * 512 : (n + 1) * 512],
                                        start=True,
                                        stop=True,
                                    )
                                et = ets[hh]
                                rsg = rsgs[hh]
                                if (i * 2 + hh) in DVE_SLOTS:
                                    nc.vector.tensor_scalar(
                                        et[:, q, :].bitcast(I16),
                                        ps[:],
                                        A_EXP,
                                        B_EXP,
                                        mybir.AluOpType.mult,
                                        mybir.AluOpType.add,
                                    )
                                    dve_pend.append((et, q, rsg))
                                else:
                                    nc.scalar.activation(
                                        out=et[:, q, :],
                                        in_=ps[:],
                                        func=mybir.ActivationFunctionType.Exp,
                                        scale=1.0 / np.sqrt(float(HS)),
                                        accum_out=rsg[:, q : q + 1],
                                    )
                            # after the first block of this group, emit the
                            # previous group's epilogue (its inputs are done;
                            # the exp stream above stays ahead in the queues)
                            if q == 0 and pending is not None:
                                emit_epilogue(pending, per_block_dma=False)
                                pending = None
                        pending = (p, grp, ets, rsgs, dve_pend)
                emit_epilogue(pending, per_block_dma=True)
    nc.compile()
    return nc


_NC_CACHE = None


def kernel(x, Wq, bq):
    global _NC_CACHE
    x = np.asarray(x, dtype=np.float32)
    Wq = np.asarray(Wq, dtype=np.float32)
    bq = np.asarray(bq, dtype=np.float32)
    assert x.shape == (B, S, D) and Wq.shape == (D, D) and bq.shape == (D,)

    if _NC_CACHE is None:
        _NC_CACHE = _build()
    nc = _NC_CACHE

    xTs = [np.ascontiguousarray(x[b].T.astype(np.float16)) for b in range(B)]
    Wq16 = Wq.astype(np.float16)
    in_maps = []
    for c in range(N_CORES):
        b, hg = divmod(c, N_CORES // B)
        h0 = hg * HEADS_PER_CORE
        wslice = Wq16[:, h0 * HS : (h0 + HEADS_PER_CORE) * HS]  # [1024, 256]
        wsw = np.ascontiguousarray(
            wslice.reshape(KK, 128, HEADS_PER_CORE * HS)
            .transpose(1, 0, 2)
            .reshape(128, KK * HEADS_PER_CORE * HS)
        )
        in_maps.append(
            {
                "xT": xTs[b],
                "WqS": wsw,
                "bqS": np.ascontiguousarray(
                    bq[h0 * HS : (h0 + HEADS_PER_CORE) * HS].reshape(2, 128).T
                ),
            }
        )

    res = run_bass_kernel_spmd(nc, in_maps, core_ids=list(range(N_CORES)))

    full = np.empty((B, H, S, S), dtype=np.float32)
    for c in range(N_CORES):
        b, hg = divmod(c, N_CORES // B)
        h0 = hg * HEADS_PER_CORE
        full[b, h0 : h0 + HEADS_PER_CORE] = np.asarray(
            res.results[c]["out"]
        ).astype(np.float32)
    return full
